# revision 40
# baseline (speedup 1.0000x reference)
"""AxialBlock on 8 Trainium2 NeuronCores (Bass/Tile).

Three axial attentions (W, H, T) over x [2,16,64,64,512] f32, summed.

Sharding: x is split over flattened (B,T) across 8 cores -> [4,64,64,512]
per core. W- and H-attention are local to a BT shard. T-attention needs
all T per (b,h,w), so the kernel reshards x to an H-shard [32,8,64,512]
with an on-device AllToAll, computes the T branch there, and AllToAll's
the branch output back to BT sharding, where a merge pass sums the
three branches plus bias.

Weights are split 769 rows/core over the host link (6.3 MB total
instead of 8x) and reassembled on-device with an AllGather.

The host link is a single shared ~45 MB/s pipe (half-duplex; total
bytes are the wall-clock bottleneck; one stream only gets ~20 MB/s so
transfers are dispatched eagerly and many-at-once). x goes over it as
8-bit codes of a cubic compander y = g^-1(x/absmax), g(u)=0.4u+0.6u^3
(~1.6x lower quantization noise than plain int8 for gaussian data);
the host encodes with a 64K-entry LUT on the top 16 bits of each f32
and the device decodes x = y*(b*y^2 + a) with runtime constants a, b
carried in the tiny `dec` input. The output comes back int8 + per-core
absmax. Compute is bf16 on the PE with fp32 PSUM accumulation.

Per 512-token tile (each branch): transpose x to [c,tok] via PE;
q^T,k^T = W^T x^T; v in [tok,c]; per (head, 128-token pack) scores
s^T = k^T q with a block-diagonal mask multiplied after exp (packs hold
2x64 or 8x16 attention groups); o = e^T v via matmul against [v | 1]
so the softmax denominator comes out as column 65; divide; transpose o;
output projection. All loops are hardware For_i loops.

Everything (bass build, NEFF compile, XLA jit, device warmup) happens
at import; kernel() runs only the data path.
"""
import sys
import os

sys.path.insert(0, "/opt/trn_rl_repo")

import numpy as np
import ml_dtypes

import concourse.bass as bass
import concourse.mybir as mybir
from concourse.tile import TileContext
from concourse.masks import make_identity
from concourse import bass2jax

import jax
from jax.sharding import Mesh, PartitionSpec, NamedSharding
from jax.experimental.shard_map import shard_map

N_CORES = 8
B, T, H, W, C = 2, 16, 64, 64, 512
NH, DK = 8, 64
BT = B * T          # 32
BTL = BT // N_CORES  # 4 bt rows per core
# cubic compander for x transport: code y=round(126.99*g^-1(x/absmax)),
# decode x_unit = DEC_A*y + DEC_B*y^3 with g(u) = 0.4u + 0.6u^3
DEC_A = 0.4 / 126.99
DEC_B = 0.6 / 126.99 ** 3
HL = H // N_CORES    # 8 h rows per core
NTOK = BTL * H * W   # 16384 tokens per core
WROWS_TOT = 6147     # 12 weight matrices (12*512) + 3 bias rows
WPC = 769            # weight rows per core (8*769 = 6152 >= 6147)
WTOT = WPC * N_CORES
bf16 = mybir.dt.bfloat16
f32 = mybir.dt.float32

_BF16 = ml_dtypes.bfloat16


def _split_waits(nc):
    """This container's walrus codegen accepts at most ONE sync-wait per
    instruction. Move extra waits onto InstNoOp carriers inserted right
    before, on the same engine queue (program order keeps semantics)."""
    n = 0
    for f in nc.m.functions:
        for blk in f.blocks:
            out = []
            for ins in blk.instructions:
                si = ins.sync_info
                if si is not None and len(si.on_wait) > 1:
                    for w in si.on_wait[:-1]:
                        nop = mybir.InstNoOp(
                            name=nc.get_next_instruction_name(), ins=[], outs=[])
                        nop.engine = ins.engine
                        nop.sync_info = mybir.SyncInfo(on_wait=[w], on_update=[])
                        nc.register_instruction(nop)
                        out.append(nop)
                        n += 1
                    si.on_wait = si.on_wait[-1:]
                out.append(ins)
            blk.instructions[:] = out
    return n


def _emit_tile(nc, pools, Wq, Wk, Wv, Wo, ident, mask, decs,
               x_loads, dst_stores):
    """One 512-token tile of one axial-attention branch.

    x_ap/dst_ap: DRAM APs shaped [128, 4, 512] (partition=token-in-pack,
    chunk=pack, free=channel). Attention groups are contiguous partition
    ranges inside each 128-token pack; `mask` (bf16 [128,1,128]) is the
    block-diagonal group mask.
    """
    sb, ps_t, ps_p, ps_s, ps_o = pools

    x_i8 = sb.tile([128, 4, 512], mybir.dt.int8, tag="x_i8")
    load_engs = (nc.sync, nc.scalar)
    for i, (sl, ap) in enumerate(x_loads):
        load_engs[i % 2].dma_start(x_i8[sl] if sl else x_i8, ap)

    # compander decode: x = a*y + b*y^3 for int8 code y, computed as
    # x = y * (b*y^2 + a) with runtime per-partition scalars
    # decs = (a, sqrt(b)).
    dec_a, dec_sb = decs
    xf = sb.tile([128, 4, 512], bf16, tag="xf")
    nc.scalar.activation(xf.rearrange("p a b -> p (a b)"),
                         x_i8.rearrange("p a b -> p (a b)"),
                         mybir.ActivationFunctionType.Copy)
    y2s = sb.tile([128, 4, 512], bf16, tag="y2s")
    nc.scalar.activation(y2s.rearrange("p a b -> p (a b)"),
                         x_i8.rearrange("p a b -> p (a b)"),
                         mybir.ActivationFunctionType.Square,
                         scale=dec_sb)
    tpoly = sb.tile([128, 4, 512], bf16, tag="tpoly")
    nc.vector.tensor_scalar_add(tpoly, y2s, dec_a)
    x_in = sb.tile([128, 4, 512], bf16, tag="x_in")
    nc.vector.tensor_tensor(x_in, xf, tpoly, mybir.AluOpType.mult)

    # x^T: [c%128, cblk, tok]
    xT = sb.tile([128, 4, 512], bf16, tag="xT")
    for k in range(4):
        for cb in range(4):
            pt = ps_t.tile([128, 128], bf16, tag="tp")
            nc.tensor.transpose(pt, x_in[:, k, cb * 128:(cb + 1) * 128], ident)
            nc.vector.tensor_copy(xT[:, cb, k * 128:(k + 1) * 128], pt)

    # q^T, k^T: [c_out%128, cblk, tok]
    qT = sb.tile([128, 4, 512], bf16, tag="qT")
    kT = sb.tile([128, 4, 512], bf16, tag="kT")
    for dst, Wmat in ((qT, Wq), (kT, Wk)):
        for cb in range(4):
            pp = ps_p.tile([128, 512], f32, tag="proj")
            for kb in range(4):
                nc.tensor.matmul(pp, Wmat[:, kb, cb * 128:(cb + 1) * 128],
                                 xT[:, kb, :], start=(kb == 0), stop=(kb == 3))
            nc.vector.tensor_copy(dst[:, cb, :], pp)

    # v in [tok, c] layout, extended with a ones column per head
    v_ext = sb.tile([128, 4, 8, 65], bf16, tag="v_ext")
    nc.vector.memset(v_ext[:, :, :, 64:65], 1.0)
    for k in range(4):
        pp = ps_p.tile([128, 512], f32, tag="proj")
        for kb in range(4):
            nc.tensor.matmul(pp, xT[:, kb, k * 128:(k + 1) * 128],
                             Wv[:, kb, :], start=(kb == 0), stop=(kb == 3))
        nc.vector.tensor_copy(
            v_ext[:, k, :, 0:64], pp.rearrange("p (h d) -> p h d", h=8))

    # attention per head; o_all in [tok, c]
    o_all = sb.tile([128, 4, 512], bf16, tag="o_all")
    for h in range(8):
        po = 64 * (h % 2)
        cbh = h // 2
        ps = ps_s.tile([128, 512], f32, tag="s")
        for k in range(4):
            nc.tensor.matmul(ps[:, k * 128:(k + 1) * 128],
                             kT[po:po + 64, cbh, k * 128:(k + 1) * 128],
                             qT[po:po + 64, cbh, k * 128:(k + 1) * 128],
                             start=True, stop=True)
        e = sb.tile([128, 4, 128], bf16, tag="e")
        nc.scalar.activation(e.rearrange("p a b -> p (a b)"), ps,
                             mybir.ActivationFunctionType.Exp, scale=0.125)
        nc.vector.tensor_tensor(e, e, mask.to_broadcast((128, 4, 128)),
                                mybir.AluOpType.mult)
        po_t = ps_o.tile([128, 4, 65], f32, tag="o")
        for k in range(4):
            nc.tensor.matmul(po_t[:, k, :], e[:, k, :], v_ext[:, k, h, :],
                             start=True, stop=True)
        csum = sb.tile([128, 4, 1], f32, tag="csum")
        nc.vector.reciprocal(csum, po_t[:, :, 64:65])
        for k in range(4):
            nc.vector.tensor_tensor(o_all[:, k, 64 * h:64 * h + 64],
                                    po_t[:, k, 0:64],
                                    csum[:, k, :].to_broadcast((128, 64)),
                                    mybir.AluOpType.mult)

    # o^T then output projection back to [tok, c]
    oT = sb.tile([128, 4, 512], bf16, tag="oT")
    for k in range(4):
        for cb in range(4):
            pt = ps_t.tile([128, 128], bf16, tag="tp")
            nc.tensor.transpose(pt, o_all[:, k, cb * 128:(cb + 1) * 128], ident)
            nc.vector.tensor_copy(oT[:, cb, k * 128:(k + 1) * 128], pt)

    out_sb = sb.tile([128, 4, 512], bf16, tag="out_sb")
    for k in range(4):
        pp = ps_p.tile([128, 512], f32, tag="proj")
        for kb in range(4):
            nc.tensor.matmul(pp, oT[:, kb, k * 128:(k + 1) * 128],
                             Wo[:, kb, :], start=(kb == 0), stop=(kb == 3))
        nc.vector.tensor_copy(out_sb[:, k, :], pp)
    store_engs = ((nc.scalar,) if len(dst_stores) <= 2 else
                  (nc.gpsimd,))
    for i, (sl, ap) in enumerate(dst_stores):
        store_engs[i % len(store_engs)].dma_start(
            ap, out_sb[sl] if sl else out_sb)


def build_nc():
    nc = bass.Bass(num_devices=N_CORES)

    x = nc.dram_tensor("x", [BTL, H, W, C], mybir.dt.int8,
                       kind="ExternalInput")
    w_in = nc.dram_tensor("w_in", [WPC, C], bf16, kind="ExternalInput")
    dec = nc.dram_tensor("dec", [1, 2], f32, kind="ExternalInput")
    out = nc.dram_tensor("out", [BTL, H, W, C], mybir.dt.int8,
                         kind="ExternalOutput")
    oscale = nc.dram_tensor("oscale", [1, 1], f32, kind="ExternalOutput")
    obuf = nc.dram_tensor("obuf", [BTL * H * W, C], bf16, kind="Internal")
    smax_d = nc.dram_tensor("smax_d", [128, 1], f32, kind="Internal")
    sc_d = nc.dram_tensor("sc_d", [1, 1], f32, kind="Internal")

    w_stage = nc.dram_tensor("w_stage", [WPC, C], bf16, kind="Internal")
    w_g = nc.dram_tensor("w_g", [WTOT, C], bf16, kind="Internal",
                         addr_space="Shared")
    wbuf = nc.dram_tensor("wbuf", [BTL * H * W, C], bf16, kind="Internal")
    hbuf = nc.dram_tensor("hbuf", [BTL * H * W, C], bf16, kind="Internal")
    a2a_xin = nc.dram_tensor("a2a_xin", [N_CORES * BTL * HL * W, C],
                             mybir.dt.int8, kind="Internal")
    a2a_xout = nc.dram_tensor("a2a_xout", [BT * HL * W, C], mybir.dt.int8,
                              kind="Internal")
    a2a_tin = nc.dram_tensor("a2a_tin", [BT * HL * W, C], bf16,
                             kind="Internal")
    a2a_tout = nc.dram_tensor("a2a_tout", [N_CORES * BTL * HL * W, C], bf16,
                              kind="Internal")

    groups = [list(range(N_CORES))]

    with TileContext(nc) as tc:
        with (
            tc.tile_pool(name="const", bufs=1) as cpool,
            tc.tile_pool(name="sb", bufs=2) as sb,
            tc.tile_pool(name="ps_t", bufs=2, space="PSUM") as ps_t,
            tc.tile_pool(name="ps_p", bufs=2, space="PSUM") as ps_p,
            tc.tile_pool(name="ps_s", bufs=2, space="PSUM") as ps_s,
            tc.tile_pool(name="ps_o", bufs=2, space="PSUM") as ps_o,
        ):
            pools = (sb, ps_t, ps_p, ps_s, ps_o)

            # broadcast weights: stage -> AllGather -> every core reads
            # core 0's block of w_g
            nc.sync.dma_start(w_stage[:], w_in[:])
            nc.gpsimd.collective_compute(
                "AllGather", mybir.AluOpType.bypass, replica_groups=groups,
                ins=[w_stage[:]], outs=[w_g[:]])

            wsb = {}
            for i, name in enumerate(
                    ("Wq_w", "Wk_w", "Wv_w", "Wo_w",
                     "Wq_h", "Wk_h", "Wv_h", "Wo_h",
                     "Wq_t", "Wk_t", "Wv_t", "Wo_t")):
                t = cpool.tile([128, 4, C], bf16, tag=f"w{i}")
                nc.sync.dma_start(
                    t, w_g[i * C:(i + 1) * C].rearrange(
                        "(kb p) n -> p kb n", p=128))
                wsb[name] = t

            # bias3 = bo_w + bo_h + bo_t, replicated across partitions
            btmp = [cpool.tile([128, C], bf16, tag=f"b{i}", name=f"btmp{i}")
                    for i in range(2)]
            bias3 = cpool.tile([128, 1, C], bf16, tag="bias3")
            nc.sync.dma_start(btmp[0], w_g[12 * C:12 * C + 1].to_broadcast((128, C)))
            nc.sync.dma_start(btmp[1], w_g[12 * C + 1:12 * C + 2].to_broadcast((128, C)))
            nc.vector.tensor_add(btmp[0], btmp[0], btmp[1])
            nc.sync.dma_start(btmp[1], w_g[12 * C + 2:12 * C + 3].to_broadcast((128, C)))
            nc.vector.tensor_add(btmp[0], btmp[0], btmp[1])
            nc.vector.tensor_copy(bias3.rearrange("p o c -> p (o c)"), btmp[0])

            ident = cpool.tile([128, 128], bf16, tag="ident")
            make_identity(nc, ident)

            # runtime compander decode constants (absmax-dependent):
            # dec = [absmax*DEC_A, sqrt(absmax*DEC_B)] broadcast to all
            # partitions.  Keeping absmax out of the weights lets the
            # weight upload start before the host has even scanned x.
            dec_a = cpool.tile([128, 1], f32, tag="dec_a")
            dec_sb = cpool.tile([128, 1], f32, tag="dec_sb")
            nc.sync.dma_start(dec_a, dec[0:1, 0:1].to_broadcast((128, 1)))
            nc.sync.dma_start(dec_sb, dec[0:1, 1:2].to_broadcast((128, 1)))

            # block-diagonal group masks (1 on diag blocks, 0 off)
            mask_wh = cpool.tile([128, 1, 128], bf16, tag="mask_wh")
            m2 = mask_wh.rearrange("p o f -> p (o f)")
            nc.vector.memset(m2, 0.0)
            nc.vector.memset(m2[0:64, 0:64], 1.0)
            nc.vector.memset(m2[64:128, 64:128], 1.0)
            mask_t = cpool.tile([128, 1, 128], bf16, tag="mask_t")
            mt_np = np.zeros((128, 128), dtype=_BF16)
            for g in range(8):
                mt_np[g * 16:(g + 1) * 16, g * 16:(g + 1) * 16] = 1
            mt_dram = nc.inline_tensor(mt_np, name="mask_t_const")
            nc.sync.dma_start(mask_t.rearrange("p o f -> p (o f)"), mt_dram[:])

            xf = x.rearrange("b h w c -> (b h w) c")
            wf = wbuf[:]
            af = a2a_xin[:]

            # ---- stage a2a_xin = x permuted [oct][bt][hl][w][c]:
            # 8 static DRAM->DRAM copies, one per h-octet ----
            for oct in range(HL):
                nc.sync.dma_start(
                    af[oct * BTL * 512:(oct + 1) * BTL * 512].rearrange(
                        "(bt r) c -> bt (r c)", bt=BTL),
                    xf.rearrange("(bt hr r) c -> hr bt (r c)", bt=BTL,
                                 hr=HL)[oct])

            # ---- W branch (groups = W rows; tokens contiguous) ----
            with tc.For_i(0, BTL * H * W, 512) as r0:
                _emit_tile(
                    nc, pools, wsb["Wq_w"], wsb["Wk_w"], wsb["Wv_w"],
                    wsb["Wo_w"], ident, mask_wh, (dec_a, dec_sb),
                    [(None, xf[bass.ds(r0, 512)].rearrange(
                        "(k p) c -> p k c", p=128))],
                    [(None, wf[bass.ds(r0, 512)].rearrange(
                        "(k p) c -> p k c", p=128))])

            # ---- H branch (groups = H columns) ----
            xh = x.rearrange("b h (wp wi) c -> wp wi h b c", wi=2)
            hh = hbuf.rearrange("(b h wp wi) c -> wp wi h b c", b=BTL, h=H, wi=2)
            with tc.For_i(0, W // 2, 1) as wp:
                xs = xh[bass.ds(wp, 1)]
                hs = hh[bass.ds(wp, 1)]
                _emit_tile(
                    nc, pools, wsb["Wq_h"], wsb["Wk_h"], wsb["Wv_h"],
                    wsb["Wo_h"], ident, mask_wh, (dec_a, dec_sb),
                    [(np.s_[0:64], xs[0, 0]), (np.s_[64:128], xs[0, 1])],
                    [(np.s_[0:64], hs[0, 0]), (np.s_[64:128], hs[0, 1])])

            # ---- x reshard: BT shard -> H shard ----
            nc.gpsimd.collective_compute(
                "AllToAll", mybir.AluOpType.bypass, replica_groups=groups,
                ins=[a2a_xin[:]], outs=[a2a_xout[:]])

            # ---- T branch on H shard (groups = T within each b) ----
            # (wl, c) are contiguous in DRAM -> merge to one 2048 dim so
            # each tile moves with a single 3-dim dynamic DMA.
            xt = a2a_xout.rearrange("r c -> (r c)").rearrange(
                "(b t hl wq wlc) -> b wq hl t wlc",
                b=B, t=T, hl=HL, wq=W // 4, wlc=4 * C)
            tt = a2a_tin.rearrange("r c -> (r c)").rearrange(
                "(b t hl wq wlc) -> b wq hl t wlc",
                b=B, t=T, hl=HL, wq=W // 4, wlc=4 * C)
            for b in range(B):
                with tc.For_i(0, W // 4, 1) as wq:
                    _emit_tile(
                        nc, pools, wsb["Wq_t"], wsb["Wk_t"], wsb["Wv_t"],
                        wsb["Wo_t"], ident, mask_t, (dec_a, dec_sb),
                        [(None, xt[b][bass.ds(wq, 1)])],
                        [(None, tt[b][bass.ds(wq, 1)])])

            # ---- T branch output back to BT sharding ----
            nc.gpsimd.collective_compute(
                "AllToAll", mybir.AluOpType.bypass, replica_groups=groups,
                ins=[a2a_tin[:]], outs=[a2a_tout[:]])

            # ---- merge: obuf = w + h + t + bias; track |out| max ----
            of = out.rearrange("b h w c -> (b h w) c")
            ob = obuf[:]
            hf = hbuf[:]
            tf = a2a_tout[:]
            stats = cpool.tile([128, 32], f32, tag="stats")
            with tc.tile_pool(name="mg", bufs=3) as mg:
                for btl in range(BTL):
                    for i in range(HL):
                        m = btl * HL + i
                        r0 = btl * H * W + i * 512
                        rt = i * 2048 + btl * 512
                        ta = mg.tile([128, 4, 512], bf16, tag="ma")
                        tb = mg.tile([128, 4, 512], bf16, tag="mb")
                        tcx = mg.tile([128, 4, 512], bf16, tag="mc")
                        nc.sync.dma_start(ta, wf[r0:r0 + 512].rearrange(
                            "(k p) c -> p k c", p=128))
                        nc.sync.dma_start(tb, hf[r0:r0 + 512].rearrange(
                            "(k p) c -> p k c", p=128))
                        nc.sync.dma_start(tcx, tf[rt:rt + 512].rearrange(
                            "(k p) c -> p k c", p=128))
                        nc.vector.tensor_add(ta, ta, tb)
                        nc.vector.tensor_add(ta, ta, tcx)
                        nc.vector.tensor_add(
                            ta, ta, bias3.to_broadcast((128, 4, 512)))
                        nc.vector.tensor_reduce(
                            stats[:, m:m + 1],
                            ta.rearrange("p a b -> p (a b)"),
                            axis=mybir.AxisListType.X, op=mybir.AluOpType.max,
                            apply_absolute_value=True)
                        nc.sync.dma_start(ob[r0:r0 + 512].rearrange(
                            "(k p) c -> p k c", p=128), ta)

                # absmax across tiles then partitions (via a DRAM bounce),
                # then quantize obuf -> int8 out with scale 126/absmax.
                pmax = cpool.tile([128, 1], f32, tag="pmax")
                nc.vector.tensor_reduce(pmax, stats,
                                        axis=mybir.AxisListType.X,
                                        op=mybir.AluOpType.max)
                nc.sync.dma_start(smax_d[:], pmax)
                prow = cpool.tile([1, 128], f32, tag="prow")
                nc.sync.dma_start(prow, smax_d.rearrange("p o -> (o p)")[None, :])
                amax = cpool.tile([1, 1], f32, tag="amax")
                nc.vector.tensor_reduce(amax, prow,
                                        axis=mybir.AxisListType.X,
                                        op=mybir.AluOpType.max)
                nc.sync.dma_start(oscale[:], amax)
                qscale = cpool.tile([1, 1], f32, tag="qscale")
                nc.vector.reciprocal(qscale, amax)
                nc.scalar.mul(qscale, qscale, 126.0)
                nc.sync.dma_start(sc_d[:], qscale)
                sc_bc = cpool.tile([128, 1], f32, tag="sc_bc")
                nc.sync.dma_start(sc_bc, sc_d.rearrange("o s -> (o s)")
                                  .to_broadcast((128, 1)))
                for btl in range(BTL):
                    for i in range(HL):
                        r0 = btl * H * W + i * 512
                        tq = mg.tile([128, 4, 512], bf16, tag="tq")
                        qi = mg.tile([128, 4, 512], mybir.dt.int8, tag="qi")
                        nc.sync.dma_start(tq, ob[r0:r0 + 512].rearrange(
                            "(k p) c -> p k c", p=128))
                        nc.scalar.activation(
                            qi.rearrange("p a b -> p (a b)"),
                            tq.rearrange("p a b -> p (a b)"),
                            mybir.ActivationFunctionType.Copy, scale=sc_bc)
                        nc.sync.dma_start(of[r0:r0 + 512].rearrange(
                            "(k p) c -> p k c", p=128), qi)

    n = _split_waits(nc)
    return nc, n


# ---------------------------------------------------------------------------
# Executor: compiled once at import; kernel() only runs the data path.
# ---------------------------------------------------------------------------
_EXEC = {}


def _setup():
    nc, nsplit = build_nc()
    bass2jax.install_neuronx_cc_hook()

    in_names, out_names, out_avals = [], [], []
    partition_name = (nc.partition_id_tensor.name
                      if nc.partition_id_tensor else None)
    for alloc in nc.m.functions[0].allocations:
        if not isinstance(alloc, mybir.MemoryLocationSet):
            continue
        name = alloc.memorylocations[0].name
        if alloc.kind == "ExternalInput":
            if name != partition_name:
                in_names.append(name)
        elif alloc.kind == "ExternalOutput":
            out_names.append(name)
            out_avals.append(jax.core.ShapedArray(
                tuple(alloc.tensor_shape), mybir.dt.np(alloc.dtype)))
    n_params, n_outs = len(in_names), len(out_avals)
    all_names = list(in_names) + out_names + (
        [partition_name] if partition_name else [])

    def _body(*args):
        operands = list(args)
        if partition_name is not None:
            operands.append(bass2jax.partition_id_tensor())
        outs = bass2jax._bass_exec_p.bind(
            *operands, out_avals=tuple(out_avals), in_names=tuple(all_names),
            out_names=tuple(out_names), lowering_input_output_aliases=(),
            sim_require_finite=True, sim_require_nnan=True, nc=nc)
        return tuple(outs)

    devices = jax.devices()[:N_CORES]
    mesh = Mesh(np.asarray(devices), ("core",))
    sharded = jax.jit(
        shard_map(_body, mesh=mesh,
                  in_specs=(PartitionSpec("core"),) * (n_params + n_outs),
                  out_specs=(PartitionSpec("core"),) * n_outs, check_rep=False),
        donate_argnums=tuple(range(n_params, n_params + n_outs)),
        keep_unused=True)

    sh = NamedSharding(mesh, PartitionSpec("core"))
    zeros_jits = []
    for av in out_avals:
        gshape = (N_CORES * av.shape[0],) + tuple(av.shape[1:])
        zeros_jits.append(jax.jit(
            lambda gs=gshape, dt=av.dtype: jax.numpy.zeros(gs, dt),
            out_shardings=sh))
    in_zero_jits = {}

    _EXEC.update(nc=nc, in_names=in_names, out_names=out_names,
                 sharded=sharded, zeros_jits=zeros_jits, sh=sh,
                 devices=list(devices), n_params=n_params,
                 out_scratch=np.zeros((BT, H, W, C), dtype=np.float32))

    # warm up the FULL data path twice: per-shard device_put ->
    # make_array -> exec -> D2H -> upcast, exactly as kernel() runs it.
    for _ in range(2):
        xz = np.zeros((BT, H, W, C), dtype=np.float32)
        wargs = {n: np.zeros((C, C), np.float32) for n in _WNAMES}
        for bn in ("bo_w", "bo_h", "bo_t"):
            wargs[bn] = np.zeros((C,), np.float32)
        xz.reshape(-1)[0] = 1.0  # nonzero absmax
        x_dev, w_dev, dec_dev = _put_inputs(xz, wargs)
        ins = {"x": x_dev, "w_in": w_dev, "dec": dec_dev}
        zouts = [zj() for zj in zeros_jits]
        res = sharded(*[ins[n] for n in in_names], *zouts)
        jax.block_until_ready(res)
        _ = _fetch_dequant(res)


_WNAMES = ("Wq_w", "Wk_w", "Wv_w", "Wo_w",
           "Wq_h", "Wk_h", "Wv_h", "Wo_h",
           "Wq_t", "Wk_t", "Wv_t", "Wo_t")


def _build_lut(absmax):
    """int8 code for every possible top-16-bit f32 pattern of x.

    Encode is then a single gather: code = LUT[x.view(u16)[1::2]].
    Each pattern is decoded at its interval midpoint (unbiased)."""
    pats = np.arange(65536, dtype=np.uint32)
    with np.errstate(invalid="ignore", over="ignore"):
        lo = (pats << np.uint32(16)).view(np.float32).astype(np.float64)
        hi = ((pats + np.uint32(1)) << np.uint32(16)).view(np.float32) \
            .astype(np.float64)
        xv = np.nan_to_num((lo + hi) * 0.5, nan=0.0,
                           posinf=absmax, neginf=-absmax)
    np.clip(xv, -absmax, absmax, out=xv)
    yg = np.linspace(-1.0, 1.0, 1 << 17)
    c3 = 0.6 * absmax
    c1 = 0.4 * absmax
    y = np.interp(xv, c1 * yg + c3 * yg ** 3, yg)
    return np.rint(y * 126.99).astype(np.int8)


def _fetch_dequant(outs):
    """Fetch the int8 output shards in two waves of four streams (four
    already saturate the tunnel) so wave A's host-side dequant runs
    while wave B is still on the wire."""
    onames = _EXEC["out_names"]
    shards = [s.data for s in outs[onames.index("out")].addressable_shards]
    half = N_CORES // 2
    for s in shards[:half]:
        s.copy_to_host_async()
    amax = np.asarray(outs[onames.index("oscale")])
    out_f = _EXEC["out_scratch"]  # pre-touched pages, reused per call

    def dq(c, r):
        np.multiply(r, np.float32(amax[c, 0] / 126.0),
                    out=out_f[c * BTL:(c + 1) * BTL], dtype=np.float32,
                    casting="unsafe")

    bufs_a = [np.asarray(s) for s in shards[:half]]  # blocks on wave A
    for s in shards[half:]:
        s.copy_to_host_async()                       # wave B on the wire
    for c, r in enumerate(bufs_a):
        dq(c, r)                                     # overlaps wave B
    for c in range(half, N_CORES):
        dq(c, np.asarray(shards[c]))
    return out_f


def _pack_w(args):
    """Global weight array [WTOT, C] bf16."""
    w_g = np.zeros((WTOT, C), dtype=_BF16)
    for i, name in enumerate(_WNAMES):
        w_g[i * C:(i + 1) * C] = np.asarray(args[name], dtype=np.float32)
    w_g[12 * C + 0] = np.asarray(args["bo_w"], dtype=np.float32)
    w_g[12 * C + 1] = np.asarray(args["bo_h"], dtype=np.float32)
    w_g[12 * C + 2] = np.asarray(args["bo_t"], dtype=np.float32)
    return w_g


def _put_inputs(x, args):
    """Stream the inputs to the devices with async device_put. A single
    transfer stream only gets ~20 MB/s through the tunnel while several
    in flight aggregate to ~45 MB/s, so dispatch everything eagerly and
    let the transfers overlap the remaining host-side encode work."""
    devices = _EXEC["devices"]
    sh = _EXEC["sh"]
    w_dev = jax.device_put(_pack_w(args), sh)  # on the wire immediately

    xr = x.reshape(N_CORES, BTL * H * W * C)
    absmax = max(float(xr.max()), -float(xr.min()))
    if absmax == 0.0:
        absmax = 1.0
    lut = _build_lut(absmax)
    dec_g = np.tile(np.array([[absmax * DEC_A,
                               np.sqrt(absmax * DEC_B)]], np.float32),
                    (N_CORES, 1))
    dec_dev = jax.device_put(dec_g, sh)

    # encode shard c, dispatch its transfer, move on: the transfers run
    # while later shards encode (they do contend for the single host
    # core, but starting transfers early still wins).
    devs = []
    for c in range(N_CORES):
        code = lut[xr[c].view(np.uint16)[1::2]]
        devs.append(jax.device_put(code.reshape(BTL, H, W, C), devices[c]))

    x_dev = jax.make_array_from_single_device_arrays(
        (BT, H, W, C), sh, devs)
    return x_dev, w_dev, dec_dev


def kernel(x,
           Wq_w, Wk_w, Wv_w, Wo_w, bo_w,
           Wq_h, Wk_h, Wv_h, Wo_h, bo_h,
           Wq_t, Wk_t, Wv_t, Wo_t, bo_t):
    args = locals()
    import time as _t
    dbg = os.environ.get("KDEBUG")
    t0 = _t.time()
    x = np.ascontiguousarray(np.asarray(x, dtype=np.float32))
    zouts = [zj() for zj in _EXEC["zeros_jits"]]
    x_dev, w_dev, dec_dev = _put_inputs(x, args)
    t1 = _t.time()

    ins = {"x": x_dev, "w_in": w_dev, "dec": dec_dev}
    global_in = [ins[name] for name in _EXEC["in_names"]]
    outs = _EXEC["sharded"](*global_in, *zouts)
    t3 = _t.time()
    out_f = _fetch_dequant(outs).reshape(B, T, H, W, C)
    t5 = _t.time()
    if dbg:
        print(f"[kdbg] put {t1-t0:.2f}s  exec {t3-t1:.2f}s  "
              f"d2h+dq {t5-t3:.2f}s")
    return out_f


if not os.environ.get("KBUILD_ONLY"):
    _setup()


if __name__ == "__main__":
    if os.environ.get("KBUILD_ONLY"):
        nc, nsplit = build_nc()
    else:
        nc = _EXEC["nc"]
        nsplit = None
    tot = sum(len(b.instructions) for f in nc.m.functions for b in f.blocks)
    print("instructions:", tot, "split waits:", nsplit)

